# revision 3
# baseline (speedup 1.0000x reference)
"""Block-sparse (block-diagonal, BLOCK=64) multi-head attention for 8 Trainium2 cores.

Sharding: the B*S = 4096 token rows are split into 8 contiguous slices of 512
tokens (attention is block-diagonal with 64-token blocks, so slices at
512-token boundaries are fully independent). Each core runs the whole
projections + attention + output projection for its 512 tokens; weights are
replicated. No collectives; host concatenates the per-core outputs.

Layout strategy (per core):
  - host passes X slices TRANSPOSED (feature-major [1024, 512]) so the kernel
    never has to transpose fp32 on-chip.
  - Q^T, K^T are produced feature-major [dout, t] (lhsT = W tile, rhs = X^T).
  - V is produced token-major [t, dout]  (lhsT = X^T tile, rhs = W tile).
  - scores for a 128-token chunk: S^T[j, i] = sum_dk K^T[dk, j] Q^T[dk, i]
    (both operands feature-major). The [128, 128] psum tile holds 4 quadrants
    of 64x64 blocks; the 2 off-diagonal quadrants are masked with -1e6 before
    exp, so exp() gives exact zeros there.
  - row sums r[i] come from a ones-vector matmul over the 128 partitions
    (off-diag quadrants are zero, so it is exactly the in-block sum).
  - P = exp(S^T)/r is normalized in SBUF, then O^T[dv, i] = V.T @ P with
    lhsT = V (token-major) -- output is feature-major, ready to be the lhsT
    of the final output projection, which emits token-major Y for DMA out.
"""

import sys

sys.path.insert(0, "/opt/trn_rl_repo")

import numpy as np

N_CORES = 8
B, S, D = 2, 2048, 1024
H, DK = 16, 64
T = (B * S) // N_CORES      # 512 tokens per core
P = 128
KO = D // P                 # 8 contraction tiles
MO = D // P                 # 8 d_out tiles
NC_CHUNKS = T // P          # 4 token chunks per core
HP = H // 2                 # 8 head pairs
NEG = -1.0e6

_cache = {}


def _build_program(dt_proj, dt_scores, dt_out, dt_r):
    import concourse.tile as tile
    from concourse import bacc, mybir

    f32 = mybir.dt.float32
    dts = {
        "f32": mybir.dt.float32,
        "f32r": mybir.dt.float32r,
        "bf16": mybir.dt.bfloat16,
    }
    dt_proj = dts[dt_proj]
    dt_scores = dts[dt_scores]
    dt_out = dts[dt_out]
    dt_r = dts[dt_r]

    def mm_ap(ap, dt):
        return ap if dt == f32 else ap.bitcast(dt)

    nc = bacc.Bacc("TRN2", target_bir_lowering=False, debug=False)

    xq_d = nc.dram_tensor("xq", [D, T], f32, kind="ExternalInput").ap()
    xk_d = nc.dram_tensor("xk", [D, T], f32, kind="ExternalInput").ap()
    xv_d = nc.dram_tensor("xv", [D, T], f32, kind="ExternalInput").ap()
    wq_d = nc.dram_tensor("wq", [MO, P, KO, P], f32, kind="ExternalInput").ap()
    wk_d = nc.dram_tensor("wk", [MO, P, KO, P], f32, kind="ExternalInput").ap()
    wv_d = nc.dram_tensor("wv", [D, D], f32, kind="ExternalInput").ap()
    wo_d = nc.dram_tensor("wo", [D, D], f32, kind="ExternalInput").ap()
    bq_d = nc.dram_tensor("bq", [P, MO], f32, kind="ExternalInput").ap()
    bk_d = nc.dram_tensor("bk", [P, MO], f32, kind="ExternalInput").ap()
    bv_d = nc.dram_tensor("bv", [D], f32, kind="ExternalInput").ap()
    bo_d = nc.dram_tensor("bo", [D], f32, kind="ExternalInput").ap()
    y_d = nc.dram_tensor("y", [T, D], f32, kind="ExternalOutput").ap()

    with tile.TileContext(nc) as tc:
        with (
            tc.tile_pool(name="singles", bufs=1) as singles,
            tc.tile_pool(name="wqk", bufs=3) as wqk_pool,
            tc.tile_pool(name="wvy", bufs=3) as wvy_pool,
            tc.tile_pool(name="p2", bufs=3) as p2_pool,
            tc.tile_pool(name="rec", bufs=2) as rec_pool,
            tc.tile_pool(name="ystage", bufs=3) as y_pool,
            tc.tile_pool(name="psproj", bufs=4, space="PSUM") as psproj,
            tc.tile_pool(name="psatt", bufs=3, space="PSUM") as psatt,
            tc.tile_pool(name="psr", bufs=1, space="PSUM") as psr,
        ):
            # ---- persistent SBUF tensors ----
            xq_sb = singles.tile([P, KO, T], f32, tag="xq")
            xk_sb = singles.tile([P, KO, T], f32, tag="xk")
            xv_sb = singles.tile([P, KO, T], f32, tag="xv")
            qT_sb = singles.tile([P, MO, T], f32, tag="qT")
            kT_sb = singles.tile([P, MO, T], f32, tag="kT")
            v_sb = singles.tile([P, NC_CHUNKS, D], f32, tag="v")
            oT_sb = singles.tile([P, MO, T], f32, tag="oT")
            bq_sb = singles.tile([P, MO], f32, tag="bq")
            bk_sb = singles.tile([P, MO], f32, tag="bk")
            bv_sb = singles.tile([P, D], f32, tag="bv")
            bo_sb = singles.tile([P, D], f32, tag="bo")
            mask_sb = singles.tile([P, P], f32, tag="mask")
            ones_sb = singles.tile([P, P], f32, tag="ones")

            nc.sync.dma_start(xq_sb[:], xq_d.rearrange("(ko p) t -> p ko t", p=P))
            nc.sync.dma_start(xk_sb[:], xk_d.rearrange("(ko p) t -> p ko t", p=P))
            nc.sync.dma_start(xv_sb[:], xv_d.rearrange("(ko p) t -> p ko t", p=P))
            nc.sync.dma_start(bq_sb[:], bq_d[:])
            nc.sync.dma_start(bk_sb[:], bk_d[:])
            nc.sync.dma_start(bv_sb[:], bv_d[None, :].to_broadcast([P, D]))
            nc.sync.dma_start(bo_sb[:], bo_d[None, :].to_broadcast([P, D]))

            nc.vector.memset(mask_sb[:], NEG)
            nc.vector.memset(mask_sb[0:64, 0:64], 0.0)
            nc.vector.memset(mask_sb[64:128, 64:128], 0.0)
            nc.vector.memset(ones_sb[:], 1.0)

            # ---- Q^T / K^T projections (feature-major out) ----
            for w_d, b_sb, dst in ((wq_d, bq_sb, qT_sb), (wk_d, bk_sb, kT_sb)):
                for m in range(MO):
                    w_sb = wqk_pool.tile([P, KO, P], f32, tag="wqk")
                    nc.sync.dma_start(w_sb[:], w_d[m])
                    ps = psproj.tile([P, T], f32, tag="psproj")
                    for ko in range(KO):
                        nc.tensor.matmul(
                            ps[:],
                            mm_ap(w_sb[:, ko, :], dt_proj),
                            mm_ap(
                                (xq_sb if dst is qT_sb else xk_sb)[:, ko, :], dt_proj
                            ),
                            start=(ko == 0),
                            stop=(ko == KO - 1),
                        )
                    nc.scalar.activation(
                        dst[:, m, :],
                        ps[:],
                        mybir.ActivationFunctionType.Identity,
                        bias=b_sb[:, m : m + 1],
                    )

            # ---- V projection (token-major out) ----
            NV = D // T  # 2 chunks of 512 along d_out
            for n in range(NV):
                ps_v = [
                    psproj.tile([P, T], f32, tag="psproj", name=f"psv_{n}_{i}")
                    for i in range(NC_CHUNKS)
                ]
                for ko in range(KO):
                    w_sb = wvy_pool.tile([P, T], f32, tag="wvy")
                    nc.sync.dma_start(
                        w_sb[:], wv_d[ko * P : (ko + 1) * P, n * T : (n + 1) * T]
                    )
                    for mt in range(NC_CHUNKS):
                        nc.tensor.matmul(
                            ps_v[mt][:],
                            mm_ap(xv_sb[:, ko, mt * P : (mt + 1) * P], dt_proj),
                            mm_ap(w_sb[:], dt_proj),
                            start=(ko == 0),
                            stop=(ko == KO - 1),
                        )
                for mt in range(NC_CHUNKS):
                    nc.vector.tensor_add(
                        v_sb[:, mt, n * T : (n + 1) * T],
                        ps_v[mt][:],
                        bv_sb[:, n * T : (n + 1) * T],
                    )

            # ---- attention: per token chunk (128 = 2 blocks) x head pair ----
            for c in range(NC_CHUNKS):
                tsl = slice(c * P, (c + 1) * P)
                for hp in range(HP):
                    h0, h1 = 2 * hp, 2 * hp + 1
                    p2 = p2_pool.tile([P, 2 * P], f32, tag="p2")
                    for s, h in ((slice(0, 64), h0), (slice(64, 128), h1)):
                        ps_s = psatt.tile([P, P], f32, tag="psatt")
                        nc.tensor.matmul(
                            ps_s[:],
                            mm_ap(kT_sb[s, hp, tsl], dt_scores),
                            mm_ap(qT_sb[s, hp, tsl], dt_scores),
                            start=True,
                            stop=True,
                        )
                        nc.vector.tensor_add(ps_s[:], ps_s[:], mask_sb[:])
                        nc.scalar.activation(
                            p2[:, (h % 2) * P : (h % 2 + 1) * P],
                            ps_s[:],
                            mybir.ActivationFunctionType.Exp,
                            scale=0.125,
                        )
                    ps_r = psr.tile([P, 2 * P], f32, tag="psr")
                    nc.tensor.matmul(
                        ps_r[:],
                        mm_ap(ones_sb[:], dt_r),
                        mm_ap(p2[:], dt_r),
                        start=True,
                        stop=True,
                    )
                    rec = rec_pool.tile([P, 2 * P], f32, tag="rec")
                    nc.vector.reciprocal(rec[:], ps_r[:])
                    nc.vector.tensor_mul(p2[:], p2[:], rec[:])
                    ps_o = psatt.tile([P, P], f32, tag="psatt")
                    for i, h in ((0, h0), (1, h1)):
                        nc.tensor.matmul(
                            ps_o[i * 64 : (i + 1) * 64, :],
                            mm_ap(v_sb[:, c, h * DK : (h + 1) * DK], dt_out),
                            mm_ap(p2[:, i * P : (i + 1) * P], dt_out),
                            start=True,
                            stop=True,
                        )
                    nc.scalar.activation(
                        oT_sb[:, hp, tsl],
                        ps_o[:],
                        mybir.ActivationFunctionType.Identity,
                    )

            # ---- output projection (token-major out) ----
            for n in range(NV):
                ps_y = [
                    psproj.tile([P, T], f32, tag="psproj", name=f"psy_{n}_{i}")
                    for i in range(NC_CHUNKS)
                ]
                for m in range(MO):
                    w_sb = wvy_pool.tile([P, T], f32, tag="wvy")
                    nc.sync.dma_start(
                        w_sb[:], wo_d[m * P : (m + 1) * P, n * T : (n + 1) * T]
                    )
                    for mt in range(NC_CHUNKS):
                        nc.tensor.matmul(
                            ps_y[mt][:],
                            mm_ap(oT_sb[:, m, mt * P : (mt + 1) * P], dt_proj),
                            mm_ap(w_sb[:], dt_proj),
                            start=(m == 0),
                            stop=(m == MO - 1),
                        )
                for mt in range(NC_CHUNKS):
                    y_sb = y_pool.tile([P, T], f32, tag="ystage")
                    nc.vector.tensor_add(
                        y_sb[:],
                        ps_y[mt][:],
                        bo_sb[:, n * T : (n + 1) * T],
                    )
                    nc.sync.dma_start(
                        y_d[mt * P : (mt + 1) * P, n * T : (n + 1) * T], y_sb[:]
                    )

    nc.compile()
    return nc


def _get_program(dtypes):
    if dtypes not in _cache:
        _cache[dtypes] = _build_program(*dtypes)
    return _cache[dtypes]


# matmul operand dtypes per stage: (projections, scores, attn-out, row-sum)
DEFAULT_DTYPES = ("f32", "f32", "f32", "f32")


def kernel(
    query,
    key,
    value,
    Wq,
    bq,
    Wk,
    bk,
    Wv,
    bv,
    Wo,
    bo,
    _dtypes=DEFAULT_DTYPES,
    _trace=False,
):
    from concourse.bass_utils import run_bass_kernel_spmd

    nc = _get_program(tuple(_dtypes))

    def pre_w(w):
        # [din, dout] -> [m, p, ko, c] tiles so each m-tile DMAs contiguously
        return np.ascontiguousarray(
            np.asarray(w, np.float32).reshape(KO, P, MO, P).transpose(2, 1, 0, 3)
        )

    def pre_b(b):
        return np.ascontiguousarray(np.asarray(b, np.float32).reshape(MO, P).T)

    q2 = np.asarray(query, np.float32).reshape(B * S, D)
    k2 = np.asarray(key, np.float32).reshape(B * S, D)
    v2 = np.asarray(value, np.float32).reshape(B * S, D)
    shared = {
        "wq": pre_w(Wq),
        "wk": pre_w(Wk),
        "wv": np.ascontiguousarray(np.asarray(Wv, np.float32)),
        "wo": np.ascontiguousarray(np.asarray(Wo, np.float32)),
        "bq": pre_b(bq),
        "bk": pre_b(bk),
        "bv": np.ascontiguousarray(np.asarray(bv, np.float32)),
        "bo": np.ascontiguousarray(np.asarray(bo, np.float32)),
    }
    in_maps = []
    for c in range(N_CORES):
        rows = slice(c * T, (c + 1) * T)
        in_maps.append(
            {
                "xq": np.ascontiguousarray(q2[rows].T),
                "xk": np.ascontiguousarray(k2[rows].T),
                "xv": np.ascontiguousarray(v2[rows].T),
                **shared,
            }
        )

    kwargs = {}
    if _trace:
        kwargs = {"trace": True}
    res = run_bass_kernel_spmd(nc, in_maps, core_ids=list(range(N_CORES)), **kwargs)
    y = np.concatenate([res.results[c]["y"] for c in range(N_CORES)], axis=0)
    out = y.reshape(B, S, D)
    if _trace:
        return out, res
    return out


# revision 4
# speedup vs baseline: 2.9199x; 2.9199x over previous
"""Block-sparse (block-diagonal, BLOCK=64) multi-head attention for 8 Trainium2 cores.

Sharding: the B*S = 4096 token rows are split into 8 contiguous slices of 512
tokens (attention is block-diagonal with 64-token blocks, so slices at
512-token boundaries are fully independent). Each core runs the whole
projections + attention + output projection for its 512 tokens; weights are
replicated. No collectives; host concatenates the per-core outputs.

Layout strategy (per core):
  - host passes X slices TRANSPOSED (feature-major [1024, 512]) so the kernel
    never has to transpose on-chip.
  - Q^T, K^T are produced feature-major [dout, t] (lhsT = W tile, rhs = X^T).
  - V is produced token-major [t, dout]  (lhsT = X^T tile, rhs = W tile).
  - scores for a 128-token chunk: S^T[j, i] = sum_dk K^T[dk, j] Q^T[dk, i]
    (both operands feature-major). The [128, 128] psum tile holds 4 quadrants
    of 64x64 blocks; the 2 off-diagonal quadrants are masked with -1e6 before
    exp, so exp() gives exact zeros there.
  - row sums r[i]: two ones-vector matmuls replicate the per-head in-block
    column sums into the matching 64-partition strips of a [128, 128] psum
    tile; reciprocal_approx_fast gives 1/r, and the normalization is folded
    into the PSUM->SBUF copy of the attention output (tensor_mul).
  - O^T[dv, i] = V.T @ P with lhsT = V (token-major) -- output is
    feature-major, ready to be the lhsT of the final output projection, which
    emits token-major Y for DMA out.

Compute dtype: matmul operands (X, W, Q/K/V, P, O) are stored in fp16 by
default -- 4x matmul throughput vs fp32 and half the DMA bytes, with fp32
PSUM accumulation everywhere. The all-f32 variant is available via
_compute="f32".
"""

import sys

sys.path.insert(0, "/opt/trn_rl_repo")

import numpy as np

N_CORES = 8
B, S, D = 2, 2048, 1024
H, DK = 16, 64
T = (B * S) // N_CORES      # 512 tokens per core
P = 128
KO = D // P                 # 8 contraction tiles
MO = D // P                 # 8 d_out tiles
NC_CHUNKS = T // P          # 4 token chunks per core
HP = H // 2                 # 8 head pairs
NEG = -1.0e6

_cache = {}


def _build_program(compute):
    import concourse.tile as tile
    from concourse import bacc, mybir

    f32 = mybir.dt.float32
    dtc = {"f32": f32, "f16": mybir.dt.float16, "bf16": mybir.dt.bfloat16}[compute]

    nc = bacc.Bacc("TRN2", target_bir_lowering=False, debug=False)

    xq_d = nc.dram_tensor("xq", [D, T], dtc, kind="ExternalInput").ap()
    xk_d = nc.dram_tensor("xk", [D, T], dtc, kind="ExternalInput").ap()
    xv_d = nc.dram_tensor("xv", [D, T], dtc, kind="ExternalInput").ap()
    wq_d = nc.dram_tensor("wq", [MO, P, KO, P], dtc, kind="ExternalInput").ap()
    wk_d = nc.dram_tensor("wk", [MO, P, KO, P], dtc, kind="ExternalInput").ap()
    wv_d = nc.dram_tensor("wv", [D, D], dtc, kind="ExternalInput").ap()
    wo_d = nc.dram_tensor("wo", [D, D], dtc, kind="ExternalInput").ap()
    bq_d = nc.dram_tensor("bq", [P, MO], f32, kind="ExternalInput").ap()
    bk_d = nc.dram_tensor("bk", [P, MO], f32, kind="ExternalInput").ap()
    bv_d = nc.dram_tensor("bv", [D], f32, kind="ExternalInput").ap()
    bo_d = nc.dram_tensor("bo", [D], f32, kind="ExternalInput").ap()
    y_d = nc.dram_tensor("y", [T, D], f32, kind="ExternalOutput").ap()

    with tile.TileContext(nc) as tc:
        with (
            tc.tile_pool(name="singles", bufs=1) as singles,
            tc.tile_pool(name="wqk", bufs=3) as wqk_pool,
            tc.tile_pool(name="wvy", bufs=3) as wvy_pool,
            tc.tile_pool(name="p2", bufs=3) as p2_pool,
            tc.tile_pool(name="rec", bufs=3) as rec_pool,
            tc.tile_pool(name="ystage", bufs=3) as y_pool,
            tc.tile_pool(name="psproj", bufs=4, space="PSUM") as psproj,
            tc.tile_pool(name="psatt", bufs=3, space="PSUM") as psatt,
            tc.tile_pool(name="psr", bufs=1, space="PSUM") as psr,
        ):
            # ---- persistent SBUF tensors ----
            xq_sb = singles.tile([P, KO, T], dtc, tag="xq")
            xk_sb = singles.tile([P, KO, T], dtc, tag="xk")
            xv_sb = singles.tile([P, KO, T], dtc, tag="xv")
            qT_sb = singles.tile([P, MO, T], dtc, tag="qT")
            kT_sb = singles.tile([P, MO, T], dtc, tag="kT")
            v_sb = singles.tile([P, NC_CHUNKS, D], dtc, tag="v")
            oT_sb = singles.tile([P, MO, T], dtc, tag="oT")
            bq_sb = singles.tile([P, MO], f32, tag="bq")
            bk_sb = singles.tile([P, MO], f32, tag="bk")
            bv_sb = singles.tile([P, D], f32, tag="bv")
            bo_sb = singles.tile([P, D], f32, tag="bo")
            mask_sb = singles.tile([P, P], f32, tag="mask")
            ones_sb = singles.tile([P, 64], dtc, tag="ones")

            nc.sync.dma_start(xq_sb[:], xq_d.rearrange("(ko p) t -> p ko t", p=P))
            nc.sync.dma_start(xk_sb[:], xk_d.rearrange("(ko p) t -> p ko t", p=P))
            nc.sync.dma_start(xv_sb[:], xv_d.rearrange("(ko p) t -> p ko t", p=P))
            nc.sync.dma_start(bq_sb[:], bq_d[:])
            nc.sync.dma_start(bk_sb[:], bk_d[:])
            nc.sync.dma_start(bv_sb[:], bv_d[None, :].to_broadcast([P, D]))
            nc.sync.dma_start(bo_sb[:], bo_d[None, :].to_broadcast([P, D]))

            nc.vector.memset(mask_sb[:], NEG)
            nc.vector.memset(mask_sb[0:64, 0:64], 0.0)
            nc.vector.memset(mask_sb[64:128, 64:128], 0.0)
            nc.vector.memset(ones_sb[:], 1.0)

            # ---- Q^T / K^T projections (feature-major out) ----
            for w_d, x_sb, b_sb, dst in (
                (wq_d, xq_sb, bq_sb, qT_sb),
                (wk_d, xk_sb, bk_sb, kT_sb),
            ):
                for m in range(MO):
                    w_sb = wqk_pool.tile([P, KO, P], dtc, tag="wqk")
                    nc.sync.dma_start(w_sb[:], w_d[m])
                    ps = psproj.tile([P, T], f32, tag="psproj")
                    for ko in range(KO):
                        nc.tensor.matmul(
                            ps[:],
                            w_sb[:, ko, :],
                            x_sb[:, ko, :],
                            start=(ko == 0),
                            stop=(ko == KO - 1),
                        )
                    nc.scalar.activation(
                        dst[:, m, :],
                        ps[:],
                        mybir.ActivationFunctionType.Identity,
                        bias=b_sb[:, m : m + 1],
                    )

            # ---- V projection (token-major out) ----
            NV = D // T  # 2 chunks of 512 along d_out
            for n in range(NV):
                ps_v = [
                    psproj.tile([P, T], f32, tag="psproj", name=f"psv_{n}_{i}")
                    for i in range(NC_CHUNKS)
                ]
                for ko in range(KO):
                    w_sb = wvy_pool.tile([P, T], dtc, tag="wvy")
                    nc.sync.dma_start(
                        w_sb[:], wv_d[ko * P : (ko + 1) * P, n * T : (n + 1) * T]
                    )
                    for mt in range(NC_CHUNKS):
                        nc.tensor.matmul(
                            ps_v[mt][:],
                            xv_sb[:, ko, mt * P : (mt + 1) * P],
                            w_sb[:],
                            start=(ko == 0),
                            stop=(ko == KO - 1),
                        )
                for mt in range(NC_CHUNKS):
                    nc.vector.tensor_add(
                        v_sb[:, mt, n * T : (n + 1) * T],
                        ps_v[mt][:],
                        bv_sb[:, n * T : (n + 1) * T],
                    )

            # ---- attention: per token chunk (128 = 2 blocks) x head pair ----
            for c in range(NC_CHUNKS):
                tsl = slice(c * P, (c + 1) * P)
                for hp in range(HP):
                    h0, h1 = 2 * hp, 2 * hp + 1
                    p2 = p2_pool.tile([P, 2 * P], dtc, tag="p2")
                    for idx, h in ((0, h0), (1, h1)):
                        s = slice((h % 2) * 64, (h % 2) * 64 + 64)
                        ps_s = psatt.tile([P, P], f32, tag="psatt")
                        nc.tensor.matmul(
                            ps_s[:],
                            kT_sb[s, hp, tsl],
                            qT_sb[s, hp, tsl],
                            start=True,
                            stop=True,
                        )
                        nc.vector.tensor_add(ps_s[:], ps_s[:], mask_sb[:])
                        nc.scalar.activation(
                            p2[:, idx * P : (idx + 1) * P],
                            ps_s[:],
                            mybir.ActivationFunctionType.Exp,
                            scale=0.125,
                        )
                    # replicated in-block column sums: head h0 -> partitions
                    # 0:64, head h1 -> partitions 64:128
                    ps_r = psr.tile([P, P], f32, tag="psr")
                    nc.tensor.matmul(
                        ps_r[0:64, :], ones_sb[:], p2[:, 0:P], start=True, stop=True
                    )
                    nc.tensor.matmul(
                        ps_r[64:128, :],
                        ones_sb[:],
                        p2[:, P : 2 * P],
                        start=True,
                        stop=True,
                    )
                    rec = rec_pool.tile([P, P], f32, tag="rec")
                    nc.vector.reciprocal_approx_fast(out=rec[:], in_=ps_r[:])
                    ps_o = psatt.tile([P, P], f32, tag="psatt")
                    for idx, h in ((0, h0), (1, h1)):
                        nc.tensor.matmul(
                            ps_o[idx * 64 : (idx + 1) * 64, :],
                            v_sb[:, c, h * DK : (h + 1) * DK],
                            p2[:, idx * P : (idx + 1) * P],
                            start=True,
                            stop=True,
                        )
                    nc.vector.tensor_mul(oT_sb[:, hp, tsl], ps_o[:], rec[:])

            # ---- output projection (token-major out) ----
            for n in range(NV):
                ps_y = [
                    psproj.tile([P, T], f32, tag="psproj", name=f"psy_{n}_{i}")
                    for i in range(NC_CHUNKS)
                ]
                for m in range(MO):
                    w_sb = wvy_pool.tile([P, T], dtc, tag="wvy")
                    nc.sync.dma_start(
                        w_sb[:], wo_d[m * P : (m + 1) * P, n * T : (n + 1) * T]
                    )
                    for mt in range(NC_CHUNKS):
                        nc.tensor.matmul(
                            ps_y[mt][:],
                            oT_sb[:, m, mt * P : (mt + 1) * P],
                            w_sb[:],
                            start=(m == 0),
                            stop=(m == MO - 1),
                        )
                for mt in range(NC_CHUNKS):
                    y_sb = y_pool.tile([P, T], f32, tag="ystage")
                    nc.vector.tensor_add(
                        y_sb[:],
                        ps_y[mt][:],
                        bo_sb[:, n * T : (n + 1) * T],
                    )
                    nc.sync.dma_start(
                        y_d[mt * P : (mt + 1) * P, n * T : (n + 1) * T], y_sb[:]
                    )

    nc.compile()
    return nc


def _get_program(compute):
    if compute not in _cache:
        _cache[compute] = _build_program(compute)
    return _cache[compute]


DEFAULT_COMPUTE = "f16"


def kernel(
    query,
    key,
    value,
    Wq,
    bq,
    Wk,
    bk,
    Wv,
    bv,
    Wo,
    bo,
    _compute=DEFAULT_COMPUTE,
    _trace=False,
):
    from concourse.bass_utils import run_bass_kernel_spmd

    nc = _get_program(_compute)
    if _compute == "bf16":
        import ml_dtypes

        npdt = ml_dtypes.bfloat16
    else:
        npdt = {"f32": np.float32, "f16": np.float16}[_compute]

    def pre_w(w):
        # [din, dout] -> [m, p, ko, c] tiles so each m-tile DMAs contiguously
        return np.ascontiguousarray(
            np.asarray(w, np.float32)
            .reshape(KO, P, MO, P)
            .transpose(2, 1, 0, 3)
            .astype(npdt)
        )

    def pre_b(b):
        return np.ascontiguousarray(np.asarray(b, np.float32).reshape(MO, P).T)

    q2 = np.asarray(query, np.float32).reshape(B * S, D)
    k2 = np.asarray(key, np.float32).reshape(B * S, D)
    v2 = np.asarray(value, np.float32).reshape(B * S, D)
    shared = {
        "wq": pre_w(Wq),
        "wk": pre_w(Wk),
        "wv": np.ascontiguousarray(np.asarray(Wv, np.float32).astype(npdt)),
        "wo": np.ascontiguousarray(np.asarray(Wo, np.float32).astype(npdt)),
        "bq": pre_b(bq),
        "bk": pre_b(bk),
        "bv": np.ascontiguousarray(np.asarray(bv, np.float32)),
        "bo": np.ascontiguousarray(np.asarray(bo, np.float32)),
    }
    in_maps = []
    for c in range(N_CORES):
        rows = slice(c * T, (c + 1) * T)
        in_maps.append(
            {
                "xq": np.ascontiguousarray(q2[rows].T.astype(npdt)),
                "xk": np.ascontiguousarray(k2[rows].T.astype(npdt)),
                "xv": np.ascontiguousarray(v2[rows].T.astype(npdt)),
                **shared,
            }
        )

    kwargs = {}
    if _trace:
        kwargs = {"trace": True}
    res = run_bass_kernel_spmd(nc, in_maps, core_ids=list(range(N_CORES)), **kwargs)
    y = np.concatenate([res.results[c]["y"] for c in range(N_CORES)], axis=0)
    out = y.reshape(B, S, D)
    if _trace:
        return out, res
    return out
